# revision 9
# baseline (speedup 1.0000x reference)
"""Bidirectional LSTM on 8 Trainium2 NeuronCores (SPMD, Bass/Tile).

Problem:  x[512,64,512] -> BiLSTM(H=512) -> Linear(1024->512) -> out[512,64,512]

Sharding: batch 4-way x direction 2-way.
  core c   (c in 0..3): forward  LSTM, batch rows [c*128, (c+1)*128)
  core c+4            : backward LSTM, same rows (host passes x time-reversed)

Per-core device program (identical NEFF on all 8 cores, different data):
  for t in 0..63:
    g  = b + x_t @ W_ih.T + h_{t-1} @ W_hh.T      PE, bf16 in / fp32 PSUM
         (layout: [128 batch partitions, 2048 gates free], gate order [g,i,f,o])
    gg = tanh(g0); i,f,o = sigmoid(g1..g3)        ACT
    c  = f*c + i*gg ; h = o*tanh(c)               DVE fp32, h cast to bf16
    hsT[:,k,t*128:+128] = h.T                     PE transpose + DVE copy
  outT = w_lin_half @ hs.T                        PE, streamed over 8192 tokens

Host: prepares transposed/bf16 operands, gathers the 8 partial outputs,
adds forward+backward partials + b_lin in fp32, restores [B,T,O] layout.
"""

import os
import sys

import numpy as np
import ml_dtypes

sys.path.insert(0, "/opt/trn_rl_repo")

import concourse.bass as bass  # noqa: E402
import concourse.tile as tile  # noqa: E402
from concourse import bacc, mybir  # noqa: E402
from concourse.bass_utils import run_bass_kernel_spmd  # noqa: E402

BF16 = ml_dtypes.bfloat16
B, T, I, H, O = 512, 64, 512, 512, 512
BL = 128        # batch rows per core
G4 = 4 * H      # 2048 gate rows
NTOK = T * BL   # 8192 tokens per core
AF = mybir.ActivationFunctionType

# our gate order [g, i, f, o]; pytorch rows are [i, f, g, o]
_PERM = np.concatenate([
    np.arange(2 * H, 3 * H),   # g (cell candidate)
    np.arange(0, H),           # i
    np.arange(H, 2 * H),       # f
    np.arange(3 * H, 4 * H),   # o
])
IG, II, IF, IO = 0, 1, 2, 3

_PROGRAM = None
_LAST_RESULTS = None


def _build_program():
    dt = mybir.dt
    nc = bacc.Bacc("TRN2", target_bir_lowering=False, debug=False)

    xT_d = nc.dram_tensor("xT", [T, 128, 4, BL], dt.bfloat16, kind="ExternalInput")
    wih_d = nc.dram_tensor("wihT", [128, 4, G4], dt.bfloat16, kind="ExternalInput")
    whh_d = nc.dram_tensor("whhT", [128, 4, G4], dt.bfloat16, kind="ExternalInput")
    bbc_d = nc.dram_tensor("bbc", [128, G4], dt.float32, kind="ExternalInput")
    wlin_d = nc.dram_tensor("wlinT", [128, 4, O], dt.bfloat16, kind="ExternalInput")
    ident_d = nc.dram_tensor("ident", [128, 128], dt.bfloat16, kind="ExternalInput")
    outT_d = nc.dram_tensor("outT", [4, 128, NTOK], dt.float32, kind="ExternalOutput")

    xap = xT_d.ap()
    oap = outT_d.ap()

    with tile.TileContext(nc) as tc:
        with (
            tc.tile_pool(name="const", bufs=1) as constp,
            tc.tile_pool(name="hs", bufs=1) as hsp,
        ):
            # ACT table warmup: load the sigmoid/tanh spline set during DMAs
            warm = constp.tile([128, 1], dt.float32)
            nc.gpsimd.memset(warm[:], 0.0)
            warm2 = constp.tile([128, 1], dt.float32)
            nc.scalar.activation(warm2[:], warm[:], AF.Sigmoid)

            wih = constp.tile([128, 4, G4], dt.bfloat16)
            wihap = wih_d.ap()
            for k in range(4):
                nc.sync.dma_start(wih[:, k, :], wihap[:, k, :])
            whh = constp.tile([128, 4, G4], dt.bfloat16)
            whhap = whh_d.ap()
            for k in range(4):
                nc.sync.dma_start(whh[:, k, :], whhap[:, k, :])
            bbc = constp.tile([128, G4], dt.float32)
            nc.sync.dma_start(bbc[:], bbc_d[:])
            wlin = constp.tile([128, 4, O], dt.bfloat16)
            nc.sync.dma_start(wlin[:], wlin_d[:])
            ident = constp.tile([128, 128], dt.bfloat16)
            nc.sync.dma_start(ident[:], ident_d[:])

            # h.T history, one tile per K-chunk: [h_k partition, t*128 + b]
            hsT = [
                hsp.tile([128, NTOK], dt.bfloat16, name=f"hsT{k}") for k in range(4)
            ]

            with (
                tc.tile_pool(name="xin", bufs=6) as xp,
                tc.tile_pool(name="gates", bufs=6, space="PSUM") as gps,
                tc.tile_pool(name="trps", bufs=2, space="PSUM") as trp,
                tc.tile_pool(name="cell", bufs=3) as cp,
                tc.tile_pool(name="acts", bufs=8) as app,
            ):
                def emit_transpose(h_bf, t):
                    tr = trp.tile([128, 512], dt.bfloat16, tag="tr")
                    for j in range(4):
                        nc.tensor.transpose(
                            tr[:, bass.ts(j, 128)], h_bf[:, bass.ts(j, 128)], ident[:]
                        )
                    for j in range(4):
                        nc.vector.tensor_copy(
                            hsT[j][:, bass.ts(t, 128)], tr[:, bass.ts(j, 128)]
                        )

                c_prev = None
                h_prev = None
                for t in range(T):
                    xt = xp.tile([128, 4, BL], dt.bfloat16, tag="xt")
                    nc.sync.dma_start(xt[:], xap[t])

                    gt = [
                        gps.tile([128, 512], dt.float32, tag="g", name=f"g{t}_{n}")
                        for n in range(4)
                    ]
                    for k in range(4):
                        for n in range(4):
                            nc.tensor.matmul(
                                gt[n][:], xt[:, k, :], wih[:, k, bass.ts(n, 512)],
                                start=(k == 0), stop=(t == 0 and k == 3),
                            )
                    if t > 0:
                        # transpose of h_{t-1} goes here: the x matmuls above
                        # cover step t-1's ACT/DVE chain latency
                        emit_transpose(h_prev, t - 1)
                        # gate-outer so each gate's PSUM tile completes early
                        # and the ACT/DVE chain pipelines across gates
                        for n in range(4):
                            for k in range(4):
                                nc.tensor.matmul(
                                    gt[n][:], hsT[k][:, bass.ts(t - 1, 128)],
                                    whh[:, k, bass.ts(n, 512)],
                                    start=False, stop=(k == 3),
                                )

                    # bias add (DVE, PSUM+SBUF -> SBUF) then activation, per
                    # gate in chain order so ACT/DVE pipeline across gates.
                    # ig runs on GPSIMD so the DVE queue stays clear for the
                    # serial fc -> c -> h tail.
                    gb = [
                        app.tile([128, 512], dt.float32, tag="gb", name=f"gb{t}_{n}")
                        for n in range(4)
                    ]
                    acts = []
                    for n, fn in ((IG, AF.Tanh), (II, AF.Sigmoid),
                                  (IF, AF.Sigmoid), (IO, AF.Sigmoid)):
                        nc.vector.tensor_add(gb[n][:], gt[n][:], bbc[:, bass.ts(n, 512)])
                        a = app.tile([128, 512], dt.float32, tag="act", name=f"act{t}_{n}")
                        nc.scalar.activation(a[:], gb[n][:], fn)
                        acts.append(a)
                    tg, i_s, f_s, o_s = acts

                    c_new = cp.tile([128, 512], dt.float32, tag="c")
                    if t == 0:
                        nc.vector.tensor_mul(c_new[:], i_s[:], tg[:])
                    else:
                        ig = cp.tile([128, 512], dt.float32, tag="ig")
                        nc.gpsimd.tensor_mul(ig[:], i_s[:], tg[:])
                        fc = cp.tile([128, 512], dt.float32, tag="fc")
                        nc.vector.tensor_mul(fc[:], f_s[:], c_prev[:])
                        nc.vector.tensor_add(c_new[:], ig[:], fc[:])
                    c_prev = c_new

                    tch = app.tile([128, 512], dt.float32, tag="tch")
                    nc.scalar.activation(tch[:], c_new[:], AF.Tanh)
                    h_bf = cp.tile([128, 512], dt.bfloat16, tag="h")
                    nc.vector.tensor_mul(h_bf[:], o_s[:], tch[:])
                    h_prev = h_bf

                emit_transpose(h_prev, T - 1)

            # linear phase: outT[m] = w_half[m*128:+128] @ hs.T
            with (
                tc.tile_pool(name="linps", bufs=4, space="PSUM") as linps,
                tc.tile_pool(name="linsb", bufs=4) as linsb,
            ):
                for m in range(4):
                    for nch in range(16):
                        ps = linps.tile([128, 512], dt.float32, tag="lps")
                        for k in range(4):
                            nc.tensor.matmul(
                                ps[:], wlin[:, k, bass.ts(m, 128)],
                                hsT[k][:, bass.ts(nch, 512)],
                                start=(k == 0), stop=(k == 3),
                            )
                        ob = linsb.tile([128, 512], dt.float32, tag="ob")
                        nc.vector.tensor_copy(ob[:], ps[:])
                        nc.sync.dma_start(oap[m, :, bass.ts(nch, 512)], ob[:])

    nc.compile()
    return nc


def _get_program():
    global _PROGRAM
    if _PROGRAM is None:
        _PROGRAM = _build_program()
    return _PROGRAM


def _prep_core_inputs(xc, w_ih, w_hh, b, w_lin_half, backward):
    # xc: [BL, T, I] fp32 batch chunk
    if backward:
        xc = xc[:, ::-1, :]
    # [T, i_k(128) partitions, k(4), b(128)]
    xT = np.ascontiguousarray(
        xc.transpose(1, 2, 0).reshape(T, 4, 128, BL).transpose(0, 2, 1, 3)
    ).astype(BF16)
    wihT = np.ascontiguousarray(
        w_ih[_PERM].T.reshape(4, 128, G4).transpose(1, 0, 2)
    ).astype(BF16)
    whhT = np.ascontiguousarray(
        w_hh[_PERM].T.reshape(4, 128, G4).transpose(1, 0, 2)
    ).astype(BF16)
    bbc = np.ascontiguousarray(
        np.broadcast_to(b[_PERM][None, :].astype(np.float32), (128, G4))
    )
    wlinT = np.ascontiguousarray(
        w_lin_half.T.reshape(4, 128, O).transpose(1, 0, 2)
    ).astype(BF16)
    ident = np.eye(128, dtype=BF16)
    return dict(xT=xT, wihT=wihT, whhT=whhT, bbc=bbc, wlinT=wlinT, ident=ident)


def kernel(x, w_ih_f, w_hh_f, b_f, w_ih_b, w_hh_b, b_b, w_lin, b_lin):
    global _LAST_RESULTS
    x = np.asarray(x, np.float32)
    w_ih_f = np.asarray(w_ih_f, np.float32)
    w_hh_f = np.asarray(w_hh_f, np.float32)
    b_f = np.asarray(b_f, np.float32)
    w_ih_b = np.asarray(w_ih_b, np.float32)
    w_hh_b = np.asarray(w_hh_b, np.float32)
    b_b = np.asarray(b_b, np.float32)
    w_lin = np.asarray(w_lin, np.float32)
    b_lin = np.asarray(b_lin, np.float32)

    nc = _get_program()
    in_maps = []
    for core in range(8):
        cidx = core % 4
        xc = x[cidx * BL:(cidx + 1) * BL]
        if core < 4:
            in_maps.append(
                _prep_core_inputs(xc, w_ih_f, w_hh_f, b_f, w_lin[:, :H], False)
            )
        else:
            in_maps.append(
                _prep_core_inputs(xc, w_ih_b, w_hh_b, b_b, w_lin[:, H:], True)
            )

    trace = bool(int(os.environ.get("LSTM_TRACE", "0")))
    res = run_bass_kernel_spmd(nc, in_maps, core_ids=list(range(8)), trace=trace)
    _LAST_RESULTS = res

    out = np.empty((B, T, O), np.float32)
    for cidx in range(4):
        pf = np.asarray(res.results[cidx]["outT"], np.float32)
        pb = np.asarray(res.results[cidx + 4]["outT"], np.float32)
        pf = pf.reshape(4, 128, T, BL).transpose(3, 2, 0, 1).reshape(BL, T, O)
        pb = pb.reshape(4, 128, T, BL).transpose(3, 2, 0, 1).reshape(BL, T, O)[:, ::-1]
        out[cidx * BL:(cidx + 1) * BL] = pf + pb + b_lin[None, None, :]
    return out


# revision 13
# speedup vs baseline: 1.3385x; 1.3385x over previous
"""Bidirectional LSTM on 8 Trainium2 NeuronCores (SPMD, Bass/Tile).

Problem:  x[512,64,512] -> BiLSTM(H=512) -> Linear(1024->512) -> out[512,64,512]

Sharding: batch 4-way x direction 2-way.
  core c   (c in 0..3): forward  LSTM, batch rows [c*128, (c+1)*128)
  core c+4            : backward LSTM, same rows (host passes x time-reversed)

Per-core device program (identical NEFF on all 8 cores, different data):
  for t in 0..63:
    g  = b + x_t @ W_ih.T + h_{t-1} @ W_hh.T      PE, bf16 in / fp32 PSUM
         (layout: [128 batch partitions, 2048 gates free], gate order [g,i,f,o])
    gg = tanh(g0); i,f,o = sigmoid(g1..g3)        ACT
    c  = f*c + i*gg ; h = o*tanh(c)               DVE fp32, h cast to bf16
    hsT[:,k,t*128:+128] = h.T                     PE transpose + DVE copy
  outT = w_lin_half @ hs.T                        PE, streamed over 8192 tokens

Host: prepares transposed/bf16 operands, gathers the 8 partial outputs,
adds forward+backward partials + b_lin in fp32, restores [B,T,O] layout.
"""

import os
import sys

import numpy as np
import ml_dtypes

sys.path.insert(0, "/opt/trn_rl_repo")

import concourse.bass as bass  # noqa: E402
import concourse.tile as tile  # noqa: E402
from concourse import bacc, mybir  # noqa: E402
from concourse.bass_utils import run_bass_kernel_spmd  # noqa: E402

BF16 = ml_dtypes.bfloat16
B, T, I, H, O = 512, 64, 512, 512, 512
BL = 128        # batch rows per core
G4 = 4 * H      # 2048 gate rows
NTOK = T * BL   # 8192 tokens per core
AF = mybir.ActivationFunctionType

# our gate order [g, i, f, o]; pytorch rows are [i, f, g, o]
_PERM = np.concatenate([
    np.arange(2 * H, 3 * H),   # g (cell candidate)
    np.arange(0, H),           # i
    np.arange(H, 2 * H),       # f
    np.arange(3 * H, 4 * H),   # o
])
IG, II, IF, IO = 0, 1, 2, 3

_PROGRAM = None
_LAST_RESULTS = None


def _build_program():
    dt = mybir.dt
    nc = bacc.Bacc("TRN2", target_bir_lowering=False, debug=False)

    xT_d = nc.dram_tensor("xT", [T, 128, 4, BL], dt.bfloat16, kind="ExternalInput")
    wih_d = nc.dram_tensor("wihT", [128, 4, G4], dt.bfloat16, kind="ExternalInput")
    whh_d = nc.dram_tensor("whhT", [128, 4, G4], dt.bfloat16, kind="ExternalInput")
    bbc_d = nc.dram_tensor("bbc", [128, G4], dt.float32, kind="ExternalInput")
    wlin_d = nc.dram_tensor("wlinT", [128, 4, O], dt.bfloat16, kind="ExternalInput")
    ident_d = nc.dram_tensor("ident", [128, 128], dt.bfloat16, kind="ExternalInput")
    outT_d = nc.dram_tensor("outT", [4, 128, NTOK], dt.float32, kind="ExternalOutput")

    xap = xT_d.ap()
    oap = outT_d.ap()

    with tile.TileContext(nc) as tc:
        with (
            tc.tile_pool(name="const", bufs=1) as constp,
            tc.tile_pool(name="hs", bufs=1) as hsp,
        ):
            # ACT table warmup: load the sigmoid/tanh spline set during DMAs
            warm = constp.tile([128, 1], dt.float32)
            nc.gpsimd.memset(warm[:], 0.0)
            warm2 = constp.tile([128, 1], dt.float32)
            nc.scalar.activation(warm2[:], warm[:], AF.Sigmoid)

            wih = constp.tile([128, 4, G4], dt.bfloat16)
            nc.sync.dma_start(wih[:], wih_d[:])
            whh = constp.tile([128, 4, G4], dt.bfloat16)
            nc.sync.dma_start(whh[:], whh_d[:])
            bbc = constp.tile([128, G4], dt.float32)
            nc.sync.dma_start(bbc[:], bbc_d[:])
            wlin = constp.tile([128, 4, O], dt.bfloat16)
            nc.sync.dma_start(wlin[:], wlin_d[:])
            ident = constp.tile([128, 128], dt.bfloat16)
            nc.sync.dma_start(ident[:], ident_d[:])

            # h.T history, one tile per K-chunk: [h_k partition, t*128 + b]
            hsT = [
                hsp.tile([128, NTOK], dt.bfloat16, name=f"hsT{k}") for k in range(4)
            ]

            with (
                tc.tile_pool(name="xin", bufs=6) as xp,
                tc.tile_pool(name="gates", bufs=5, space="PSUM") as gps,
                tc.tile_pool(name="trps", bufs=1, space="PSUM") as trp,
                tc.tile_pool(name="linps", bufs=2, space="PSUM") as linps,
                tc.tile_pool(name="cell", bufs=3) as cp,
                tc.tile_pool(name="acts", bufs=8) as app,
                tc.tile_pool(name="linsb", bufs=4) as linsb,
            ):
                def emit_transpose(h_bf, t):
                    tr = trp.tile([128, 512], dt.bfloat16, tag="tr")
                    for j in range(4):
                        nc.tensor.transpose(
                            tr[:, bass.ts(j, 128)], h_bf[:, bass.ts(j, 128)], ident[:]
                        )
                    for j in range(4):
                        nc.vector.tensor_copy(
                            hsT[j][:, bass.ts(t, 128)], tr[:, bass.ts(j, 128)]
                        )

                def emit_linear(nch):
                    # outT[m] partial for token chunk nch (PE-idle filler)
                    for m in range(4):
                        ps = linps.tile(
                            [128, 512], dt.float32, tag="lps", name=f"lin{nch}_{m}"
                        )
                        for k in range(4):
                            nc.tensor.matmul(
                                ps[:], wlin[:, k, bass.ts(m, 128)],
                                hsT[k][:, bass.ts(nch, 512)],
                                start=(k == 0), stop=(k == 3),
                            )
                        ob = linsb.tile(
                            [128, 512], dt.float32, tag="ob", name=f"ob{nch}_{m}"
                        )
                        nc.scalar.copy(ob[:], ps[:])
                        nc.sync.dma_start(oap[m, :, bass.ts(nch, 512)], ob[:])

                c_prev = None
                h_prev = None
                for t in range(T):
                    xt = xp.tile([128, 4, BL], dt.bfloat16, tag="xt")
                    nc.sync.dma_start(xt[:], xap[t])

                    gt = [
                        gps.tile([128, 512], dt.float32, tag="g", name=f"g{t}_{n}")
                        for n in range(4)
                    ]
                    for k in range(4):
                        for n in range(4):
                            nc.tensor.matmul(
                                gt[n][:], xt[:, k, :], wih[:, k, bass.ts(n, 512)],
                                start=(k == 0), stop=(t == 0 and k == 3),
                            )
                    if t > 0:
                        # transpose of h_{t-1} goes here: the x matmuls above
                        # cover step t-1's ACT/DVE chain latency
                        emit_transpose(h_prev, t - 1)
                        # PE-idle filler: linear partials for a finished chunk
                        if t % 4 == 0 and t >= 8:
                            emit_linear((t - 8) // 4)
                        # gate-outer so each gate's PSUM tile completes early
                        # and the ACT/DVE chain pipelines across gates
                        for n in range(4):
                            for k in range(4):
                                nc.tensor.matmul(
                                    gt[n][:], hsT[k][:, bass.ts(t - 1, 128)],
                                    whh[:, k, bass.ts(n, 512)],
                                    start=False, stop=(k == 3),
                                )

                    # bias add (DVE, PSUM+SBUF -> SBUF) then activation; DVE
                    # queue ordered so the serial ig/fc/c/h tail isn't stuck
                    # behind the o-gate bias add
                    gb = [
                        app.tile([128, 512], dt.float32, tag="gb", name=f"gb{t}_{n}")
                        for n in range(4)
                    ]
                    acts = {}
                    for n, fn in ((IG, AF.Tanh), (II, AF.Sigmoid), (IF, AF.Sigmoid)):
                        nc.vector.tensor_add(gb[n][:], gt[n][:], bbc[:, bass.ts(n, 512)])
                        a = app.tile([128, 512], dt.float32, tag="act", name=f"act{t}_{n}")
                        nc.scalar.activation(a[:], gb[n][:], fn)
                        acts[n] = a
                    tg, i_s, f_s = acts[IG], acts[II], acts[IF]

                    c_new = cp.tile([128, 512], dt.float32, tag="c")
                    if t == 0:
                        nc.vector.tensor_add(gb[IO][:], gt[IO][:], bbc[:, bass.ts(IO, 512)])
                        o_s = app.tile([128, 512], dt.float32, tag="act", name=f"act{t}_o")
                        nc.scalar.activation(o_s[:], gb[IO][:], AF.Sigmoid)
                        nc.vector.tensor_mul(c_new[:], i_s[:], tg[:])
                    else:
                        ig = cp.tile([128, 512], dt.float32, tag="ig")
                        nc.vector.tensor_mul(ig[:], i_s[:], tg[:])
                        nc.vector.tensor_add(gb[IO][:], gt[IO][:], bbc[:, bass.ts(IO, 512)])
                        o_s = app.tile([128, 512], dt.float32, tag="act", name=f"act{t}_o")
                        nc.scalar.activation(o_s[:], gb[IO][:], AF.Sigmoid)
                        fc = cp.tile([128, 512], dt.float32, tag="fc")
                        nc.vector.tensor_mul(fc[:], f_s[:], c_prev[:])
                        nc.vector.tensor_add(c_new[:], ig[:], fc[:])
                    c_prev = c_new

                    tch = app.tile([128, 512], dt.float32, tag="tch")
                    nc.scalar.activation(tch[:], c_new[:], AF.Tanh)
                    h_bf = cp.tile([128, 512], dt.bfloat16, tag="h")
                    nc.vector.tensor_mul(h_bf[:], o_s[:], tch[:])
                    h_prev = h_bf

                emit_transpose(h_prev, T - 1)
                emit_linear(14)
                emit_linear(15)

    nc.compile()
    return nc


def _get_program():
    global _PROGRAM
    if _PROGRAM is None:
        _PROGRAM = _build_program()
    return _PROGRAM


def _prep_core_inputs(xc, w_ih, w_hh, b, w_lin_half, backward):
    # xc: [BL, T, I] fp32 batch chunk
    if backward:
        xc = xc[:, ::-1, :]
    # [T, i_k(128) partitions, k(4), b(128)]
    xT = np.ascontiguousarray(
        xc.transpose(1, 2, 0).reshape(T, 4, 128, BL).transpose(0, 2, 1, 3)
    ).astype(BF16)
    wihT = np.ascontiguousarray(
        w_ih[_PERM].T.reshape(4, 128, G4).transpose(1, 0, 2)
    ).astype(BF16)
    whhT = np.ascontiguousarray(
        w_hh[_PERM].T.reshape(4, 128, G4).transpose(1, 0, 2)
    ).astype(BF16)
    bbc = np.ascontiguousarray(
        np.broadcast_to(b[_PERM][None, :].astype(np.float32), (128, G4))
    )
    wlinT = np.ascontiguousarray(
        w_lin_half.T.reshape(4, 128, O).transpose(1, 0, 2)
    ).astype(BF16)
    ident = np.eye(128, dtype=BF16)
    return dict(xT=xT, wihT=wihT, whhT=whhT, bbc=bbc, wlinT=wlinT, ident=ident)


def kernel(x, w_ih_f, w_hh_f, b_f, w_ih_b, w_hh_b, b_b, w_lin, b_lin):
    global _LAST_RESULTS
    x = np.asarray(x, np.float32)
    w_ih_f = np.asarray(w_ih_f, np.float32)
    w_hh_f = np.asarray(w_hh_f, np.float32)
    b_f = np.asarray(b_f, np.float32)
    w_ih_b = np.asarray(w_ih_b, np.float32)
    w_hh_b = np.asarray(w_hh_b, np.float32)
    b_b = np.asarray(b_b, np.float32)
    w_lin = np.asarray(w_lin, np.float32)
    b_lin = np.asarray(b_lin, np.float32)

    nc = _get_program()
    in_maps = []
    for core in range(8):
        cidx = core % 4
        xc = x[cidx * BL:(cidx + 1) * BL]
        if core < 4:
            in_maps.append(
                _prep_core_inputs(xc, w_ih_f, w_hh_f, b_f, w_lin[:, :H], False)
            )
        else:
            in_maps.append(
                _prep_core_inputs(xc, w_ih_b, w_hh_b, b_b, w_lin[:, H:], True)
            )

    trace = bool(int(os.environ.get("LSTM_TRACE", "0")))
    res = run_bass_kernel_spmd(nc, in_maps, core_ids=list(range(8)), trace=trace)
    _LAST_RESULTS = res

    out = np.empty((B, T, O), np.float32)
    for cidx in range(4):
        pf = np.asarray(res.results[cidx]["outT"], np.float32)
        pb = np.asarray(res.results[cidx + 4]["outT"], np.float32)
        pf = pf.reshape(4, 128, T, BL).transpose(3, 2, 0, 1).reshape(BL, T, O)
        pb = pb.reshape(4, 128, T, BL).transpose(3, 2, 0, 1).reshape(BL, T, O)[:, ::-1]
        out[cidx * BL:(cidx + 1) * BL] = pf + pb + b_lin[None, None, :]
    return out


# revision 16
# speedup vs baseline: 1.3489x; 1.0078x over previous
"""Bidirectional LSTM on 8 Trainium2 NeuronCores (SPMD, Bass/Tile).

Problem:  x[512,64,512] -> BiLSTM(H=512) -> Linear(1024->512) -> out[512,64,512]

Sharding: batch 4-way x direction 2-way.
  core c   (c in 0..3): forward  LSTM, batch rows [c*128, (c+1)*128)
  core c+4            : backward LSTM, same rows (host passes x time-reversed)

Per-core device program (identical NEFF on all 8 cores, different data):
  for t in 0..63:
    g  = b + x_t @ W_ih.T + h_{t-1} @ W_hh.T      PE, bf16 in / fp32 PSUM
         (layout: [128 batch partitions, 2048 gates free], gate order [g,i,f,o])
    gg = tanh(g0); i,f,o = sigmoid(g1..g3)        ACT
    c  = f*c + i*gg ; h = o*tanh(c)               DVE fp32, h cast to bf16
    hsT[:,k,t*128:+128] = h.T                     PE transpose + DVE copy
  outT = w_lin_half @ hs.T                        PE, streamed over 8192 tokens

Host: prepares transposed/bf16 operands, gathers the 8 partial outputs,
adds forward+backward partials + b_lin in fp32, restores [B,T,O] layout.
"""

import os
import sys

import numpy as np
import ml_dtypes

sys.path.insert(0, "/opt/trn_rl_repo")

import concourse.bass as bass  # noqa: E402
import concourse.tile as tile  # noqa: E402
from concourse import bacc, mybir  # noqa: E402
from concourse.bass_utils import run_bass_kernel_spmd  # noqa: E402

BF16 = ml_dtypes.bfloat16
B, T, I, H, O = 512, 64, 512, 512, 512
BL = 128        # batch rows per core
G4 = 4 * H      # 2048 gate rows
NTOK = T * BL   # 8192 tokens per core
AF = mybir.ActivationFunctionType

# our gate order [g, i, f, o]; pytorch rows are [i, f, g, o]
_PERM = np.concatenate([
    np.arange(2 * H, 3 * H),   # g (cell candidate)
    np.arange(0, H),           # i
    np.arange(H, 2 * H),       # f
    np.arange(3 * H, 4 * H),   # o
])
IG, II, IF, IO = 0, 1, 2, 3

_PROGRAM = None
_LAST_RESULTS = None


def _build_program():
    dt = mybir.dt
    nc = bacc.Bacc("TRN2", target_bir_lowering=False, debug=False)

    xT_d = nc.dram_tensor("xT", [T, 128, 4, BL], dt.bfloat16, kind="ExternalInput")
    wih_d = nc.dram_tensor("wihT", [128, 4, G4], dt.bfloat16, kind="ExternalInput")
    whh_d = nc.dram_tensor("whhT", [128, 4, G4], dt.bfloat16, kind="ExternalInput")
    bbc_d = nc.dram_tensor("bbc", [128, G4], dt.float32, kind="ExternalInput")
    wlin_d = nc.dram_tensor("wlinT", [128, 4, O], dt.bfloat16, kind="ExternalInput")
    ident_d = nc.dram_tensor("ident", [128, 128], dt.bfloat16, kind="ExternalInput")
    outT_d = nc.dram_tensor("outT", [4, 128, NTOK], dt.float32, kind="ExternalOutput")

    xap = xT_d.ap()
    oap = outT_d.ap()

    with tile.TileContext(nc) as tc:
        with (
            tc.tile_pool(name="const", bufs=1) as constp,
            tc.tile_pool(name="hs", bufs=1) as hsp,
        ):
            # ACT table warmup: load the sigmoid/tanh spline set during DMAs
            warm = constp.tile([128, 1], dt.float32)
            nc.gpsimd.memset(warm[:], 0.0)
            warm2 = constp.tile([128, 1], dt.float32)
            nc.scalar.activation(warm2[:], warm[:], AF.Sigmoid)

            # ordered by first use; x loads go on gpsimd's DMA queue so the
            # first timesteps don't queue behind 5.5MB of weights
            wih = constp.tile([128, 4, G4], dt.bfloat16)
            nc.sync.dma_start(wih[:], wih_d[:])
            bbc = constp.tile([128, G4], dt.float32)
            nc.sync.dma_start(bbc[:], bbc_d[:])
            whh = constp.tile([128, 4, G4], dt.bfloat16)
            nc.sync.dma_start(whh[:], whh_d[:])
            ident = constp.tile([128, 128], dt.bfloat16)
            nc.sync.dma_start(ident[:], ident_d[:])
            wlin = constp.tile([128, 4, O], dt.bfloat16)
            nc.sync.dma_start(wlin[:], wlin_d[:])

            # h.T history, one tile per K-chunk: [h_k partition, t*128 + b]
            hsT = [
                hsp.tile([128, NTOK], dt.bfloat16, name=f"hsT{k}") for k in range(4)
            ]

            with (
                tc.tile_pool(name="xin", bufs=6) as xp,
                tc.tile_pool(name="gates", bufs=5, space="PSUM") as gps,
                tc.tile_pool(name="trps", bufs=1, space="PSUM") as trp,
                tc.tile_pool(name="linps", bufs=2, space="PSUM") as linps,
                tc.tile_pool(name="cell", bufs=3) as cp,
                tc.tile_pool(name="acts", bufs=8) as app,
                tc.tile_pool(name="linsb", bufs=4) as linsb,
            ):
                def emit_transpose(h_bf, t):
                    tr = trp.tile([128, 512], dt.bfloat16, tag="tr")
                    for j in range(4):
                        nc.tensor.transpose(
                            tr[:, bass.ts(j, 128)], h_bf[:, bass.ts(j, 128)], ident[:]
                        )
                    for j in range(4):
                        nc.vector.tensor_copy(
                            hsT[j][:, bass.ts(t, 128)], tr[:, bass.ts(j, 128)]
                        )

                def emit_linear(nch):
                    # outT[m] partial for token chunk nch (PE-idle filler)
                    for m in range(4):
                        ps = linps.tile(
                            [128, 512], dt.float32, tag="lps", name=f"lin{nch}_{m}"
                        )
                        for k in range(4):
                            nc.tensor.matmul(
                                ps[:], wlin[:, k, bass.ts(m, 128)],
                                hsT[k][:, bass.ts(nch, 512)],
                                start=(k == 0), stop=(k == 3),
                            )
                        ob = linsb.tile(
                            [128, 512], dt.float32, tag="ob", name=f"ob{nch}_{m}"
                        )
                        nc.scalar.copy(ob[:], ps[:])
                        nc.sync.dma_start(oap[m, :, bass.ts(nch, 512)], ob[:])

                c_prev = None
                h_prev = None
                for t in range(T):
                    xt = xp.tile([128, 4, BL], dt.bfloat16, tag="xt")
                    nc.gpsimd.dma_start(xt[:], xap[t])

                    gt = [
                        gps.tile([128, 512], dt.float32, tag="g", name=f"g{t}_{n}")
                        for n in range(4)
                    ]
                    for k in range(4):
                        for n in range(4):
                            nc.tensor.matmul(
                                gt[n][:], xt[:, k, :], wih[:, k, bass.ts(n, 512)],
                                start=(k == 0), stop=(t == 0 and k == 3),
                            )
                    if t > 0:
                        # transpose of h_{t-1} goes here: the x matmuls above
                        # cover step t-1's ACT/DVE chain latency
                        emit_transpose(h_prev, t - 1)
                        # PE-idle filler: linear partials for a finished chunk
                        if t % 4 == 0 and t >= 8:
                            emit_linear((t - 8) // 4)
                        # gate-outer so each gate's PSUM tile completes early
                        # and the ACT/DVE chain pipelines across gates
                        for n in range(4):
                            for k in range(4):
                                nc.tensor.matmul(
                                    gt[n][:], hsT[k][:, bass.ts(t - 1, 128)],
                                    whh[:, k, bass.ts(n, 512)],
                                    start=False, stop=(k == 3),
                                )

                    # bias add (DVE, PSUM+SBUF -> SBUF) then activation; DVE
                    # queue ordered so the serial ig/fc/c/h tail isn't stuck
                    # behind the o-gate bias add
                    gb = [
                        app.tile([128, 512], dt.float32, tag="gb", name=f"gb{t}_{n}")
                        for n in range(4)
                    ]
                    acts = {}
                    for n, fn in ((IG, AF.Tanh), (II, AF.Sigmoid), (IF, AF.Sigmoid)):
                        nc.vector.tensor_add(gb[n][:], gt[n][:], bbc[:, bass.ts(n, 512)])
                        a = app.tile([128, 512], dt.float32, tag="act", name=f"act{t}_{n}")
                        nc.scalar.activation(a[:], gb[n][:], fn)
                        acts[n] = a
                    tg, i_s, f_s = acts[IG], acts[II], acts[IF]

                    c_new = cp.tile([128, 512], dt.float32, tag="c")
                    if t == 0:
                        nc.vector.tensor_add(gb[IO][:], gt[IO][:], bbc[:, bass.ts(IO, 512)])
                        o_s = app.tile([128, 512], dt.float32, tag="act", name=f"act{t}_o")
                        nc.scalar.activation(o_s[:], gb[IO][:], AF.Sigmoid)
                        nc.vector.tensor_mul(c_new[:], i_s[:], tg[:])
                    else:
                        ig = cp.tile([128, 512], dt.float32, tag="ig")
                        nc.vector.tensor_mul(ig[:], i_s[:], tg[:])
                        nc.vector.tensor_add(gb[IO][:], gt[IO][:], bbc[:, bass.ts(IO, 512)])
                        o_s = app.tile([128, 512], dt.float32, tag="act", name=f"act{t}_o")
                        nc.scalar.activation(o_s[:], gb[IO][:], AF.Sigmoid)
                        fc = cp.tile([128, 512], dt.float32, tag="fc")
                        nc.vector.tensor_mul(fc[:], f_s[:], c_prev[:])
                        nc.vector.tensor_add(c_new[:], ig[:], fc[:])
                    c_prev = c_new

                    tch = app.tile([128, 512], dt.float32, tag="tch")
                    nc.scalar.activation(tch[:], c_new[:], AF.Tanh)
                    h_bf = cp.tile([128, 512], dt.bfloat16, tag="h")
                    nc.vector.tensor_mul(h_bf[:], o_s[:], tch[:])
                    h_prev = h_bf

                emit_linear(14)
                emit_transpose(h_prev, T - 1)
                emit_linear(15)

    nc.compile()
    return nc


def _get_program():
    global _PROGRAM
    if _PROGRAM is None:
        _PROGRAM = _build_program()
    return _PROGRAM


def _prep_core_inputs(xc, w_ih, w_hh, b, w_lin_half, backward):
    # xc: [BL, T, I] fp32 batch chunk
    if backward:
        xc = xc[:, ::-1, :]
    # [T, i_k(128) partitions, k(4), b(128)]
    xT = np.ascontiguousarray(
        xc.transpose(1, 2, 0).reshape(T, 4, 128, BL).transpose(0, 2, 1, 3)
    ).astype(BF16)
    wihT = np.ascontiguousarray(
        w_ih[_PERM].T.reshape(4, 128, G4).transpose(1, 0, 2)
    ).astype(BF16)
    whhT = np.ascontiguousarray(
        w_hh[_PERM].T.reshape(4, 128, G4).transpose(1, 0, 2)
    ).astype(BF16)
    bbc = np.ascontiguousarray(
        np.broadcast_to(b[_PERM][None, :].astype(np.float32), (128, G4))
    )
    wlinT = np.ascontiguousarray(
        w_lin_half.T.reshape(4, 128, O).transpose(1, 0, 2)
    ).astype(BF16)
    ident = np.eye(128, dtype=BF16)
    return dict(xT=xT, wihT=wihT, whhT=whhT, bbc=bbc, wlinT=wlinT, ident=ident)


def kernel(x, w_ih_f, w_hh_f, b_f, w_ih_b, w_hh_b, b_b, w_lin, b_lin):
    global _LAST_RESULTS
    x = np.asarray(x, np.float32)
    w_ih_f = np.asarray(w_ih_f, np.float32)
    w_hh_f = np.asarray(w_hh_f, np.float32)
    b_f = np.asarray(b_f, np.float32)
    w_ih_b = np.asarray(w_ih_b, np.float32)
    w_hh_b = np.asarray(w_hh_b, np.float32)
    b_b = np.asarray(b_b, np.float32)
    w_lin = np.asarray(w_lin, np.float32)
    b_lin = np.asarray(b_lin, np.float32)

    nc = _get_program()
    in_maps = []
    for core in range(8):
        cidx = core % 4
        xc = x[cidx * BL:(cidx + 1) * BL]
        if core < 4:
            in_maps.append(
                _prep_core_inputs(xc, w_ih_f, w_hh_f, b_f, w_lin[:, :H], False)
            )
        else:
            in_maps.append(
                _prep_core_inputs(xc, w_ih_b, w_hh_b, b_b, w_lin[:, H:], True)
            )

    trace = bool(int(os.environ.get("LSTM_TRACE", "0")))
    res = run_bass_kernel_spmd(nc, in_maps, core_ids=list(range(8)), trace=trace)
    _LAST_RESULTS = res

    out = np.empty((B, T, O), np.float32)
    for cidx in range(4):
        pf = np.asarray(res.results[cidx]["outT"], np.float32)
        pb = np.asarray(res.results[cidx + 4]["outT"], np.float32)
        pf = pf.reshape(4, 128, T, BL).transpose(3, 2, 0, 1).reshape(BL, T, O)
        pb = pb.reshape(4, 128, T, BL).transpose(3, 2, 0, 1).reshape(BL, T, O)[:, ::-1]
        out[cidx * BL:(cidx + 1) * BL] = pf + pb + b_lin[None, None, :]
    return out


# revision 18
# speedup vs baseline: 1.3502x; 1.0009x over previous
"""Bidirectional LSTM on 8 Trainium2 NeuronCores (SPMD, Bass/Tile).

Problem:  x[512,64,512] -> BiLSTM(H=512) -> Linear(1024->512) -> out[512,64,512]

Sharding: batch 4-way x direction 2-way.
  core c   (c in 0..3): forward  LSTM, batch rows [c*128, (c+1)*128)
  core c+4            : backward LSTM, same rows (host passes x time-reversed)

Per-core device program (identical NEFF on all 8 cores, different data):
  for t in 0..63:
    g  = b + x_t @ W_ih.T + h_{t-1} @ W_hh.T      PE, bf16 in / fp32 PSUM
         (layout: [128 batch partitions, 2048 gates free], gate order [g,i,f,o])
    gg = tanh(g0); i,f,o = sigmoid(g1..g3)        ACT
    c  = f*c + i*gg ; h = o*tanh(c)               DVE fp32, h cast to bf16
    hsT[:,k,t*128:+128] = h.T                     PE transpose + DVE copy
  outT = w_lin_half @ hs.T                        PE, streamed over 8192 tokens

Host: prepares transposed/bf16 operands, gathers the 8 partial outputs,
adds forward+backward partials + b_lin in fp32, restores [B,T,O] layout.
"""

import os
import sys

import numpy as np
import ml_dtypes

sys.path.insert(0, "/opt/trn_rl_repo")

import concourse.bass as bass  # noqa: E402
import concourse.tile as tile  # noqa: E402
from concourse import bacc, mybir  # noqa: E402
from concourse.bass_utils import run_bass_kernel_spmd  # noqa: E402

BF16 = ml_dtypes.bfloat16
B, T, I, H, O = 512, 64, 512, 512, 512
BL = 128        # batch rows per core
G4 = 4 * H      # 2048 gate rows
NTOK = T * BL   # 8192 tokens per core
AF = mybir.ActivationFunctionType

# our gate order [g, i, f, o]; pytorch rows are [i, f, g, o]
_PERM = np.concatenate([
    np.arange(2 * H, 3 * H),   # g (cell candidate)
    np.arange(0, H),           # i
    np.arange(H, 2 * H),       # f
    np.arange(3 * H, 4 * H),   # o
])
IG, II, IF, IO = 0, 1, 2, 3

_PROGRAM = None
_LAST_RESULTS = None


def _build_program():
    dt = mybir.dt
    nc = bacc.Bacc("TRN2", target_bir_lowering=False, debug=False)

    xT_d = nc.dram_tensor("xT", [T, 128, 4, BL], dt.bfloat16, kind="ExternalInput")
    wih_d = nc.dram_tensor("wihT", [128, 4, G4], dt.bfloat16, kind="ExternalInput")
    whh_d = nc.dram_tensor("whhT", [128, 4, G4], dt.bfloat16, kind="ExternalInput")
    bbc_d = nc.dram_tensor("bbc", [128, G4], dt.float32, kind="ExternalInput")
    wlin_d = nc.dram_tensor("wlinT", [128, 4, O], dt.bfloat16, kind="ExternalInput")
    ident_d = nc.dram_tensor("ident", [128, 128], dt.bfloat16, kind="ExternalInput")
    outT_d = nc.dram_tensor("outT", [4, 128, NTOK], dt.float32, kind="ExternalOutput")

    xap = xT_d.ap()
    oap = outT_d.ap()

    with tile.TileContext(nc) as tc:
        with (
            tc.tile_pool(name="const", bufs=1) as constp,
            tc.tile_pool(name="hs", bufs=1) as hsp,
        ):
            # ACT table warmup: load the sigmoid/tanh spline set during DMAs
            warm = constp.tile([128, 1], dt.float32)
            nc.gpsimd.memset(warm[:], 0.0)
            warm2 = constp.tile([128, 1], dt.float32)
            nc.scalar.activation(warm2[:], warm[:], AF.Sigmoid)

            # ordered by first use; x loads go on gpsimd's DMA queue so the
            # first timesteps don't queue behind 5.5MB of weights
            wih = constp.tile([128, 4, G4], dt.bfloat16)
            nc.sync.dma_start(wih[:, 0:2, :], wih_d.ap()[:, 0:2, :])
            nc.scalar.dma_start(wih[:, 2:4, :], wih_d.ap()[:, 2:4, :])
            bbc = constp.tile([128, G4], dt.float32)
            nc.sync.dma_start(bbc[:], bbc_d[:])
            whh = constp.tile([128, 4, G4], dt.bfloat16)
            nc.sync.dma_start(whh[:], whh_d[:])
            ident = constp.tile([128, 128], dt.bfloat16)
            nc.sync.dma_start(ident[:], ident_d[:])
            wlin = constp.tile([128, 4, O], dt.bfloat16)
            nc.sync.dma_start(wlin[:], wlin_d[:])

            # h.T history, one tile per K-chunk: [h_k partition, t*128 + b]
            hsT = [
                hsp.tile([128, NTOK], dt.bfloat16, name=f"hsT{k}") for k in range(4)
            ]

            with (
                tc.tile_pool(name="xin", bufs=6) as xp,
                tc.tile_pool(name="gates", bufs=5, space="PSUM") as gps,
                tc.tile_pool(name="trps", bufs=1, space="PSUM") as trp,
                tc.tile_pool(name="linps", bufs=2, space="PSUM") as linps,
                tc.tile_pool(name="cell", bufs=3) as cp,
                tc.tile_pool(name="acts", bufs=8) as app,
                tc.tile_pool(name="linsb", bufs=4) as linsb,
            ):
                def emit_transpose(h_bf, t):
                    tr = trp.tile([128, 512], dt.bfloat16, tag="tr")
                    for j in range(4):
                        nc.tensor.transpose(
                            tr[:, bass.ts(j, 128)], h_bf[:, bass.ts(j, 128)], ident[:]
                        )
                    for j in range(4):
                        nc.vector.tensor_copy(
                            hsT[j][:, bass.ts(t, 128)], tr[:, bass.ts(j, 128)]
                        )

                def emit_linear(nch):
                    # outT[m] partial for token chunk nch (PE-idle filler)
                    for m in range(4):
                        ps = linps.tile(
                            [128, 512], dt.float32, tag="lps", name=f"lin{nch}_{m}"
                        )
                        for k in range(4):
                            nc.tensor.matmul(
                                ps[:], wlin[:, k, bass.ts(m, 128)],
                                hsT[k][:, bass.ts(nch, 512)],
                                start=(k == 0), stop=(k == 3),
                            )
                        ob = linsb.tile(
                            [128, 512], dt.float32, tag="ob", name=f"ob{nch}_{m}"
                        )
                        nc.scalar.copy(ob[:], ps[:])
                        nc.sync.dma_start(oap[m, :, bass.ts(nch, 512)], ob[:])

                c_prev = None
                h_prev = None
                for t in range(T):
                    xt = xp.tile([128, 4, BL], dt.bfloat16, tag="xt")
                    nc.gpsimd.dma_start(xt[:], xap[t])

                    gt = [
                        gps.tile([128, 512], dt.float32, tag="g", name=f"g{t}_{n}")
                        for n in range(4)
                    ]
                    for k in range(4):
                        for n in range(4):
                            nc.tensor.matmul(
                                gt[n][:], xt[:, k, :], wih[:, k, bass.ts(n, 512)],
                                start=(k == 0), stop=(t == 0 and k == 3),
                            )
                    if t > 0:
                        # transpose of h_{t-1} goes here: the x matmuls above
                        # cover step t-1's ACT/DVE chain latency
                        emit_transpose(h_prev, t - 1)
                        # PE-idle filler: linear partials for a finished chunk
                        if t % 4 == 0 and t >= 8:
                            emit_linear((t - 8) // 4)
                        # gate-outer so each gate's PSUM tile completes early
                        # and the ACT/DVE chain pipelines across gates
                        for n in range(4):
                            for k in range(4):
                                nc.tensor.matmul(
                                    gt[n][:], hsT[k][:, bass.ts(t - 1, 128)],
                                    whh[:, k, bass.ts(n, 512)],
                                    start=False, stop=(k == 3),
                                )

                    # bias add (DVE, PSUM+SBUF -> SBUF) then activation; DVE
                    # queue ordered so the serial ig/fc/c/h tail isn't stuck
                    # behind the o-gate bias add
                    gb = [
                        app.tile([128, 512], dt.float32, tag="gb", name=f"gb{t}_{n}")
                        for n in range(4)
                    ]
                    acts = {}
                    for n, fn in ((IG, AF.Tanh), (II, AF.Sigmoid), (IF, AF.Sigmoid)):
                        nc.vector.tensor_add(gb[n][:], gt[n][:], bbc[:, bass.ts(n, 512)])
                        a = app.tile([128, 512], dt.float32, tag="act", name=f"act{t}_{n}")
                        nc.scalar.activation(a[:], gb[n][:], fn)
                        acts[n] = a
                    tg, i_s, f_s = acts[IG], acts[II], acts[IF]

                    c_new = cp.tile([128, 512], dt.float32, tag="c")
                    if t == 0:
                        nc.vector.tensor_add(gb[IO][:], gt[IO][:], bbc[:, bass.ts(IO, 512)])
                        o_s = app.tile([128, 512], dt.float32, tag="act", name=f"act{t}_o")
                        nc.scalar.activation(o_s[:], gb[IO][:], AF.Sigmoid)
                        nc.vector.tensor_mul(c_new[:], i_s[:], tg[:])
                    else:
                        ig = cp.tile([128, 512], dt.float32, tag="ig")
                        nc.vector.tensor_mul(ig[:], i_s[:], tg[:])
                        nc.vector.tensor_add(gb[IO][:], gt[IO][:], bbc[:, bass.ts(IO, 512)])
                        o_s = app.tile([128, 512], dt.float32, tag="act", name=f"act{t}_o")
                        nc.scalar.activation(o_s[:], gb[IO][:], AF.Sigmoid)
                        fc = cp.tile([128, 512], dt.float32, tag="fc")
                        nc.vector.tensor_mul(fc[:], f_s[:], c_prev[:])
                        nc.vector.tensor_add(c_new[:], ig[:], fc[:])
                    c_prev = c_new

                    tch = app.tile([128, 512], dt.float32, tag="tch")
                    nc.scalar.activation(tch[:], c_new[:], AF.Tanh)
                    h_bf = cp.tile([128, 512], dt.bfloat16, tag="h")
                    nc.vector.tensor_mul(h_bf[:], o_s[:], tch[:])
                    h_prev = h_bf

                emit_linear(14)
                emit_transpose(h_prev, T - 1)
                emit_linear(15)

    nc.compile()
    return nc


def _get_program():
    global _PROGRAM
    if _PROGRAM is None:
        _PROGRAM = _build_program()
    return _PROGRAM


def _prep_core_inputs(xc, w_ih, w_hh, b, w_lin_half, backward):
    # xc: [BL, T, I] fp32 batch chunk
    if backward:
        xc = xc[:, ::-1, :]
    # [T, i_k(128) partitions, k(4), b(128)]
    xT = np.ascontiguousarray(
        xc.transpose(1, 2, 0).reshape(T, 4, 128, BL).transpose(0, 2, 1, 3)
    ).astype(BF16)
    wihT = np.ascontiguousarray(
        w_ih[_PERM].T.reshape(4, 128, G4).transpose(1, 0, 2)
    ).astype(BF16)
    whhT = np.ascontiguousarray(
        w_hh[_PERM].T.reshape(4, 128, G4).transpose(1, 0, 2)
    ).astype(BF16)
    bbc = np.ascontiguousarray(
        np.broadcast_to(b[_PERM][None, :].astype(np.float32), (128, G4))
    )
    wlinT = np.ascontiguousarray(
        w_lin_half.T.reshape(4, 128, O).transpose(1, 0, 2)
    ).astype(BF16)
    ident = np.eye(128, dtype=BF16)
    return dict(xT=xT, wihT=wihT, whhT=whhT, bbc=bbc, wlinT=wlinT, ident=ident)


def kernel(x, w_ih_f, w_hh_f, b_f, w_ih_b, w_hh_b, b_b, w_lin, b_lin):
    global _LAST_RESULTS
    x = np.asarray(x, np.float32)
    w_ih_f = np.asarray(w_ih_f, np.float32)
    w_hh_f = np.asarray(w_hh_f, np.float32)
    b_f = np.asarray(b_f, np.float32)
    w_ih_b = np.asarray(w_ih_b, np.float32)
    w_hh_b = np.asarray(w_hh_b, np.float32)
    b_b = np.asarray(b_b, np.float32)
    w_lin = np.asarray(w_lin, np.float32)
    b_lin = np.asarray(b_lin, np.float32)

    nc = _get_program()
    in_maps = []
    for core in range(8):
        cidx = core % 4
        xc = x[cidx * BL:(cidx + 1) * BL]
        if core < 4:
            in_maps.append(
                _prep_core_inputs(xc, w_ih_f, w_hh_f, b_f, w_lin[:, :H], False)
            )
        else:
            in_maps.append(
                _prep_core_inputs(xc, w_ih_b, w_hh_b, b_b, w_lin[:, H:], True)
            )

    trace = bool(int(os.environ.get("LSTM_TRACE", "0")))
    res = run_bass_kernel_spmd(nc, in_maps, core_ids=list(range(8)), trace=trace)
    _LAST_RESULTS = res

    out = np.empty((B, T, O), np.float32)
    for cidx in range(4):
        pf = np.asarray(res.results[cidx]["outT"], np.float32)
        pb = np.asarray(res.results[cidx + 4]["outT"], np.float32)
        pf = pf.reshape(4, 128, T, BL).transpose(3, 2, 0, 1).reshape(BL, T, O)
        pb = pb.reshape(4, 128, T, BL).transpose(3, 2, 0, 1).reshape(BL, T, O)[:, ::-1]
        out[cidx * BL:(cidx + 1) * BL] = pf + pb + b_lin[None, None, :]
    return out
